# revision 36
# baseline (speedup 1.0000x reference)
"""LocalLinOSS Trainium2 kernel — 8-core SPMD, sequence-sharded, hT-resident.

Model structure (reference): embedding lookup -> 4 sequential blocks; within a
timestep, block i reads the running hidden h (reset to x_t each step), so the
only cross-time recurrence is the per-block diagonal state
    ns_t = coeff (.) ns_{t-1} + in_to_state @ LN(h_t).
The model therefore decomposes into 4 sequential layer passes over the whole
sequence, each = big matmuls over L (parallel) + a first-order linear scan
(hardware tensor_tensor_scan), followed by the [L,D]@[D,V] output projection.

Sharding: L=4096 split into 8 chunks of T=512 (one per core). Per layer, each
core computes a local scan with zero initial state, exchanges the 8 chunk
final states (1KB payload) via ncfw AllGather, combines them into its carry-in
using host-precomputed decay powers, and applies the carry as a correction
through the s2h matmul: s2h @ ns = s2h @ ns_local + s2h @ A where
A[s,t] = c_s^{t+1} * carry_s. The two s2h @ ns_local matmuls and the scans are
issued before the exchange so they overlap it.

Layout: the hidden state lives TRANSPOSED the whole time: hT[d] = [128 D-half
rows, T=512 time cols] f32r, d in {0,1}. This kills all per-layer transposes:
 - LN is folded into the matmuls: u = (win @ hT  -  w1 (x) mu_row) (.) r_row,
   where mu/r = per-timestep mean/rstd. The rank-1 mean-subtract term is a
   K=1 matmul accumulated into the same PSUM; the rstd scale rides the
   PSUM->SBUF vector op (r broadcast across partitions via gpsimd
   partition_broadcast — DVE ops reject partition-stride-0 APs and can read
   only one PSUM operand). Same fold for the ddiag (direct) term.
 - LN stats come from PE column-sums (ones-column f32r matmuls over hT and
   hT^2); rstd = reciprocal_approx_fast(Sqrt(var+eps)) — scalar Rsqrt is
   gated in bass, and the Sqrt/Gelu activation tables don't coexist, so
   dummy activations hoist the 1.28us table swaps into scalar-idle windows.
 - delta = outp_W' @ mixed comes out of PSUM already in [D, T] layout, so the
   residual is a plain vector add (bf16-cast fold on the last layer, which is
   processed in T-halves so the vocab projection starts on the first half).
Producers feeding f32r matmuls must write f32r-rounded APs (BIR verifier
enforces it) — hence native-f32r tiles with .bitcast(f32) reads on DVE ops.

Output projection: per-core [T,512]x[256,V] in bf16 (fp32 accumulate), bf16
stores. Stores are issued as 64-partition transfers: a 128-partition DMA fans
out across two HW-DGE queues whose completion tracking is unsound under
buffer reuse (see the optimize_sems note in tile.py; observed as partitions
64-127 corrupting under schedule-dependent races), and rotate over the
sync/gpsimd queues only — scalar-queue stores hit the same corruption first.

Timing notes (measured on this terminal): the ncfw first-collective wall is
~76-92us per NEFF execution (a minimal 1-AllGather kernel runs 92us), so AG0
completes at ~77-86us no matter what; later AGs cost ~5-7us each incl skew.
The part is power-throttled (ham windows cap PE util at 50-81% for most of
the run), so the PE sustains only ~1.15-1.2 GHz: keep-warm filler matmuls
are net NEGATIVE (they burn throttle budget) and were removed; cost-model
full-speed estimates do not apply. Run-to-run variance is +-7us (wall,
throttle phase, inter-core skew). HW exec time ~215-228us vs 230us for the
previous (transpose-heavy, warm-filled) kernel.
"""
import os
import sys
sys.path.insert(0, "/opt/trn_rl_repo")
import numpy as np
import concourse.bass as bass
import concourse.bacc as bacc
import concourse.mybir as mybir
import concourse.tile as tile
from concourse.bass_utils import run_bass_kernel_spmd

L, D, S, NB, V = 4096, 256, 256, 4, 8000
NC = 8
T = L // NC            # 512 timesteps per core
P = 128
NT = T // P            # 4 T-tiles per core
NVC = 16               # projection V chunks
VC = V // NVC          # 500
f32 = mybir.dt.float32
f32r = mybir.dt.float32r
bf16 = mybir.dt.bfloat16
i32 = mybir.dt.int32
AF = mybir.ActivationFunctionType
OP = mybir.AluOpType
AX = mybir.AxisListType

WARM0 = 8              # startup PE-warm matmuls (512-col bf16)
NWARM = 0              # exchange-wait PE-warm matmuls
L0WARM = 0            # extra warm filling the first-collective wall
WGELU = 0              # fill the gelu wait between A-matmuls and residual
WRES = 0               # fill the stats wait at end of layer

_cache = {}


def _build(flags):
    key = flags
    if (nc_cached := _cache.get(key)) is not None:
        return nc_cached
    use_outb, use_ubias, use_opb = flags
    dbg = bool(os.environ.get("BASS_DBG_DUMP"))
    dbg_l = int(os.environ.get("BASS_DBG_LAYER", "0"))
    key = flags + (dbg, dbg_l)
    nc = bacc.Bacc("TRN2", target_bir_lowering=False, debug=False,
                   enable_asserts=True, num_devices=NC)

    def din(name, shape, dtype=f32):
        return nc.dram_tensor(name, shape, dtype, kind="ExternalInput").ap()

    tok_idx = din("tok_idx", [P, NT], i32)
    tok_tab = din("tok_tab", [V, D])
    posT_in = din("posT_in", [P, 2 * T])   # pre-transposed pos embed, 2 halves
    # consts blob: id_f32 (P) | coef (NB*2) | dbias (NB*2) | opb (NB*2)
    cblob_in = din("cblob_in", [P, P + NB * 6])
    # rank-1 rows for the LN fold: -w1 | -dprime, per (layer, half): [1, NB*4*P]
    rows_in = din("rows_in", [1, NB * 4 * P])
    ubias_in = din("ubias_in", [1, NB * 2 * P], bf16)  # W_in' @ ln_b lhsT rows
    onesT = din("onesT", [1, T], bf16)
    ones8 = din("ones8", [8, 1])
    wmat_in = din("wmat_in", [8, NB * S])      # per-core carry weights
    # per-layer weight blob: win (4P) | s2h (4P) | outp (4P) | ddiag (2P), f32
    wl_in = din("wl_in", [NB, P, 14 * P])
    # per-layer decay powers: cpow st0,st1 | crev st0,st1  (f32)
    cpw_in = din("cpw_in", [NB, P, 4 * T])
    outwt_in = din("outwt_in", [2, P, V], bf16)
    outb_in = din("outb_in", [1, V], bf16)
    out_d = nc.dram_tensor("out", [T, V], bf16, kind="ExternalOutput").ap()
    if dbg:
        dbg_d = nc.dram_tensor("dbg", [12, P, T], f32,
                               kind="ExternalOutput").ap()
        dbg2_d = nc.dram_tensor("dbg2", [2, P, T], bf16,
                                kind="ExternalOutput").ap()

    with tile.TileContext(nc) as tc:
        with tc.tile_pool(name="const", bufs=1) as cst, \
             tc.tile_pool(name="wts", bufs=1) as wts, \
             tc.tile_pool(name="work", bufs=1) as wk, \
             tc.tile_pool(name="lay", bufs=1) as lay, \
             tc.tile_pool(name="psum", bufs=1, space="PSUM") as ps, \
             tc.tile_pool(name="stage", bufs=1) as stg, \
             tc.tile_pool(name="dram", bufs=1, space="DRAM") as dram:

            # ---- PE self-warm source: memset, no DMA dependency ----
            warm = cst.tile([P, T], bf16)
            nc.gpsimd.memset(warm[:], 0.0)
            eps11 = cst.tile([1, 1], f32)
            nc.gpsimd.memset(eps11[:], 1e-5)
            scrap = cst.tile([1, 1], f32)          # dummy act target
            nc.gpsimd.memset(scrap[:], 1.0)

            def warm_mms(n, tag):
                if n == 0:
                    return
                w_ps = ps.tile([P, T], f32, tag="pp", bufs=8, name=f"warm{tag}")
                for w in range(n):
                    nc.tensor.matmul(w_ps[:], warm[:, :P], warm[:],
                                     start=True, stop=True)

            # ---- loads: small control data on sync/scalar; embedding gather
            # gets DMA bandwidth priority, big weight blobs queue behind it ----
            ti_sb = wk.tile([P, NT], i32)
            nc.sync.dma_start(ti_sb[:], tok_idx)
            cb_sb = cst.tile([P, P + NB * 6], f32)
            nc.sync.dma_start(cb_sb[:], cblob_in)
            coef_sb = cb_sb[:, P:P + NB * 2]
            db_sb = cb_sb[:, P + NB * 2:P + NB * 4]
            ob_sb = cb_sb[:, P + NB * 4:P + NB * 6]
            id_r = cst.tile([P, P], f32r)
            nc.sync.dma_start(id_r[:], cblob_in[:, 0:P].bitcast(f32r))
            id_f = cb_sb[:, 0:P]
            posT_sb = wk.tile([P, 2 * T], f32)
            nc.sync.dma_start(posT_sb[:], posT_in)
            rows_sb = cst.tile([1, NB * 4 * P], f32r)
            nc.scalar.dma_start(rows_sb[:], rows_in.bitcast(f32r))
            w1_sb = rows_sb[:, 0:NB * 2 * P]       # -w1 rows (layer, half)
            dp_sb = rows_sb[:, NB * 2 * P:]        # -dprime rows
            ones8_sb = cst.tile([8, 1], f32)
            nc.scalar.dma_start(ones8_sb[:], ones8)
            wm_sb = cst.tile([8, NB * S], f32)
            nc.scalar.dma_start(wm_sb[:], wmat_in)
            if use_ubias:
                ub_sb = cst.tile([1, NB * 2 * P], bf16)
                nc.scalar.dma_start(ub_sb[:], ubias_in)
                onesT_sb = cst.tile([1, T], bf16)
                nc.scalar.dma_start(onesT_sb[:], onesT)
            if use_outb:
                outb_sb = cst.tile([1, V], bf16)
                nc.scalar.dma_start(outb_sb[:], outb_in)
                ones1_sb = cst.tile([1, P], bf16)
                nc.scalar.dma_start(ones1_sb[:], onesT[:, :P])

            # preload the sqrt act table for layer 0's stats; the gelu table
            # gets loaded during layer 0's exchange wait
            nc.scalar.activation(scrap[:], eps11[:], AF.Sqrt)

            # PE warm-up stream: starts as soon as memset lands and keeps the
            # clock ramping while the embedding gather runs
            warm_mms(WARM0, "w0")

            # ---- embedding gather (bandwidth priority) ----
            h0 = wk.tile([P, NT, D], f32)
            for ct in range(NT):
                nc.gpsimd.indirect_dma_start(
                    out=h0[:, ct, :], out_offset=None, in_=tok_tab,
                    in_offset=bass.IndirectOffsetOnAxis(
                        ap=ti_sb[:, ct:ct + 1], axis=0))

            # big weight loads, queued on gpsimd behind the gathers
            wl_sb, cpw_sb = [], []
            for i in range(NB):
                wl_sb.append(wts.tile([P, 14 * P], f32r, name=f"wl{i}"))
                nc.gpsimd.dma_start(wl_sb[i][:], wl_in[i].bitcast(f32r))
                cpw_sb.append(wts.tile([P, 4 * T], f32, name=f"cpw{i}"))
                nc.gpsimd.dma_start(cpw_sb[i][:], cpw_in[i])
            win_sb = [wl_sb[i][:, 0:4 * P] for i in range(NB)]
            s2h_sb = [wl_sb[i][:, 4 * P:8 * P] for i in range(NB)]
            outp_sb = [wl_sb[i][:, 8 * P:12 * P] for i in range(NB)]
            ddiag_sb = [wl_sb[i][:, 12 * P:14 * P] for i in range(NB)]
            outwt_sb = [wts.tile([P, V], bf16, name=f"outwt{d}") for d in range(2)]
            for d in range(2):
                nc.gpsimd.dma_start(outwt_sb[d][:], outwt_in[d])

            # ---- transpose embeddings to [D, T] and add posT ----
            hT = [wk.tile([P, T], f32r, name=f"hT{d}") for d in range(2)]
            ht_ps = [ps.tile([P, T], f32, tag="pp", bufs=8, name=f"ht_ps{d}")
                     for d in range(2)]
            for ct in range(NT):
                for d in range(2):
                    nc.tensor.transpose(ht_ps[d][:, ct * P:(ct + 1) * P],
                                        h0[:, ct, d * P:(d + 1) * P], id_f)
            for d in range(2):
                nc.vector.tensor_tensor(hT[d][:], ht_ps[d][:],
                                        posT_sb[:, d * T:(d + 1) * T], op=OP.add)

            # a [128,1] f32r ones column for column-sum matmuls
            onescol = cst.tile([P, 1], f32r)
            nc.gpsimd.memset(onescol[:].bitcast(f32), 1.0)

            hsT = [None, None]

            # ---- 4 sequential layer passes ----
            for i in range(NB):
                last = i == NB - 1
                # 1. stats: column sums of hT and hT^2 via ones-matmuls
                sq0 = lay.tile([P, T], f32r, tag="sq0")
                sq1 = lay.tile([P, T], f32r, tag="sq1")
                nc.scalar.activation(sq0[:], hT[0][:].bitcast(f32), AF.Square)
                nc.vector.tensor_tensor(sq1[:], hT[1][:].bitcast(f32),
                                        hT[1][:].bitcast(f32), op=OP.mult)
                cs_ps = ps.tile([1, T], f32, tag="pp", bufs=8, name="cs_ps")
                cq_ps = ps.tile([1, T], f32, tag="pp", bufs=8, name="cq_ps")
                for d in range(2):
                    nc.tensor.matmul(cs_ps[:], onescol[:], hT[d][:],
                                     start=(d == 0), stop=(d == 1))
                nc.tensor.matmul(cq_ps[:], onescol[:], sq0[:],
                                 start=True, stop=False)
                nc.tensor.matmul(cq_ps[:], onescol[:], sq1[:],
                                 start=False, stop=True)
                # rows: mean, veps, r=rsqrt(veps), rmu=r*mean
                mean = lay.tile([1, T], f32r, tag="mean")
                ms = lay.tile([1, T], f32, tag="ms")
                veps = lay.tile([1, T], f32, tag="veps")
                sd = lay.tile([1, T], f32, tag="sd")
                r_row = lay.tile([1, T], f32, tag="r_row")
                nc.vector.tensor_scalar_mul(mean[:], cs_ps[:], 1.0 / D)
                nc.scalar.activation(ms[:], cs_ps[:], AF.Square,
                                     scale=1.0 / D)
                nc.vector.scalar_tensor_tensor(veps[:], cq_ps[:], 1.0 / D,
                                               ms[:], op0=OP.mult,
                                               op1=OP.subtract)
                # rstd = recip(sqrt(var+eps)): sqrt on scalar (table preloaded
                # by a dummy), single-instruction DVE approx reciprocal
                nc.scalar.activation(sd[:], veps[:], AF.Sqrt, bias=eps11[:])
                nc.vector.reciprocal_approx_fast(r_row[:], sd[:])
                # broadcast r_row to all partitions in SBUF (DVE ops can't
                # take partition-stride-0 operands, nor two PSUM inputs)
                r_sb = lay.tile([P, T], f32, tag="r_sb")
                nc.gpsimd.partition_broadcast(r_sb[:], r_row[:])

                # 2. u = (win @ hT - w1 (x) rmu) (.) r_row ; the chunk summary
                # (last scan state) = sum_k crev_k u_k via fused accumulate
                u = [lay.tile([P, T], f32, tag=f"u{st}", name=f"u{st}")
                     for st in range(2)]
                last2 = lay.tile([P, 4], f32, tag="last2")
                scrT = sq0  # dead after the cq colsum; reuse as accum target
                u_ps = [None, None]
                for st in range(2):
                    u_ps[st] = ps.tile([P, T], f32, tag="pp", bufs=8,
                                       name=f"u_ps{st}")
                    for kt in range(2):
                        nc.tensor.matmul(
                            u_ps[st][:],
                            win_sb[i][:, (kt * 2 + st) * P:(kt * 2 + st + 1) * P],
                            hT[kt][:], start=(kt == 0), stop=False)
                    if use_ubias:
                        nc.tensor.matmul(
                            u_ps[st][:],
                            ub_sb[:, (i * 2 + st) * P:(i * 2 + st + 1) * P],
                            onesT_sb[:], start=False, stop=False)
                    nc.tensor.matmul(
                        u_ps[st][:],
                        w1_sb[:, (i * 2 + st) * P:(i * 2 + st + 1) * P],
                        mean[:], start=False, stop=True)
                    # scale by r on the way out of PSUM
                    nc.vector.tensor_tensor(
                        u[st][:], u_ps[st][:], r_sb[:], op=OP.mult)
                    nc.vector.scalar_tensor_tensor(
                        scrT[:], u[st][:], 1.0,
                        cpw_sb[i][:, (2 + st) * T:(3 + st) * T],
                        op0=OP.bypass, op1=OP.mult,
                        accum_out=last2[:, st:st + 1])
                if dbg and i == dbg_l:
                    nc.sync.dma_start(dbg_d[0], r_sb[:])
                    nc.sync.dma_start(dbg_d[1], u[0][:])
                    nc.sync.dma_start(dbg_d[2], u[1][:])
                # 3. export chunk-final states (AllGather via shared DRAM)
                lt_ps = ps.tile([2, P], f32, tag="pp", bufs=8, name="lt_ps")
                nc.tensor.transpose(lt_ps[:], last2[:, 0:2], id_f)
                exp_sb = lay.tile([2, P], f32, tag="exp")
                nc.scalar.activation(exp_sb[:], lt_ps[:], AF.Identity)
                ag_in = dram.tile([2, P], f32, name=f"ag_in{i}")
                ag_out = dram.tile([NC, 2, P], f32, name=f"ag_out{i}",
                                   addr_space="Shared")
                nc.scalar.dma_start(ag_in[:], exp_sb[:])
                # swap the act table back to gelu while the scalar engine
                # would otherwise idle in the exchange wait
                nc.scalar.activation(scrap[:], eps11[:], AF.Gelu_apprx_tanh)
                nc.gpsimd.collective_compute(
                    "AllGather", OP.bypass, replica_groups=[list(range(NC))],
                    ins=[ag_in[:]], outs=[ag_out[:]])
                # 4. overlap the AG: local scans + carry-independent partials
                ns1 = [lay.tile([P, T], f32r, tag=f"ns1{st}", name=f"ns1{st}")
                       for st in range(2)]
                for st in range(2):
                    cb = coef_sb[:, i * 2 + st:i * 2 + st + 1].to_broadcast((P, T))
                    nc.vector.tensor_tensor_scan(ns1[st][:], cb,
                                                 u[st][:], 0.0,
                                                 op0=OP.mult, op1=OP.add)
                # dd = (ddiag @ hT - dprime (x) rmu) (.) r_row
                dd = [lay.tile([P, T], f32r, tag=f"dd{d}", name=f"dd{d}")
                      for d in range(2)]
                for d in range(2):
                    dd_ps = ps.tile([P, T], f32, tag="pp", bufs=8, name=f"dd_ps{d}")
                    nc.tensor.matmul(dd_ps[:],
                                     ddiag_sb[i][:, d * P:(d + 1) * P],
                                     hT[d][:], start=True, stop=False)
                    nc.tensor.matmul(dd_ps[:],
                                     dp_sb[:, (i * 2 + d) * P:(i * 2 + d + 1) * P],
                                     mean[:], start=False, stop=True)
                    nc.vector.tensor_tensor(dd[d][:], dd_ps[:],
                                            r_sb[:], op=OP.mult)
                if dbg and i == dbg_l:
                    nc.sync.dma_start(dbg_d[3], ns1[0][:].bitcast(f32))
                    nc.sync.dma_start(dbg_d[4], ns1[1][:].bitcast(f32))
                    nc.sync.dma_start(dbg_d[9], dd[0][:].bitcast(f32))
                # m_ps = s2h @ ns1 + dd (via identity matmul); left open for
                # the post-carry s2h @ A accumulate
                m_ps = [None, None]
                for d in range(2):
                    m_ps[d] = ps.tile([P, T], f32, tag="pp", bufs=8, name=f"m_ps{d}")
                    for st in range(2):
                        nc.tensor.matmul(m_ps[d][:],
                                         s2h_sb[i][:, (st * 2 + d) * P:(st * 2 + d + 1) * P],
                                         ns1[st][:], start=(st == 0), stop=False)
                    nc.tensor.matmul(m_ps[d][:], id_r[:], dd[d][:],
                                     start=False, stop=False)
                # PE keep-warm through the exchange wait
                warm_mms(NWARM + (L0WARM if i == 0 else 0), f"c{i}")
                # 5. combine the gathered states into the carry
                A = dd  # dd dead after its id-matmul into m_ps
                gath = lay.tile([8, S], f32, tag="gath")
                nc.scalar.dma_start(gath[:],
                                    ag_out[:].rearrange("c a b -> c (a b)"))
                q = lay.tile([8, S], f32, tag="q")
                nc.vector.tensor_tensor(q[:], wm_sb[:, i * S:(i + 1) * S],
                                        gath[:], op=OP.mult)
                for st in range(2):
                    c_ps = ps.tile([P, 1], f32, tag="pp", bufs=8,
                                   name=f"c_ps{st}")
                    nc.tensor.matmul(c_ps[:], q[:, st * P:(st + 1) * P],
                                     ones8_sb[:], start=True, stop=True)
                    nc.vector.tensor_scalar_mul(
                        A[st][:],
                        cpw_sb[i][:, st * T:(st + 1) * T], c_ps[:, 0:1])
                # 6.-8. finish mixed, delta, residual. On the last layer,
                # process T-halves so the vocab projection can start on the
                # first half while the second is still in flight.
                # (sq tiles are dead after the colsums/accum and are f32r
                # throughout, so the gelu output reuses them.)
                mixed = [sq0[:], sq1[:]]
                d_ps = [None, None]
                for d2 in range(2):
                    d_ps[d2] = ps.tile([P, T], f32, tag="pp", bufs=8,
                                       name=f"d_ps{d2}")
                if last:
                    for d2 in range(2):
                        hsT[d2] = wk.tile([P, T], bf16, name=f"hsT{d2}")
                halves = [slice(0, T)] if not last else \
                    [slice(0, T // 2), slice(T // 2, T)]
                for hi, sl in enumerate(halves):
                    for d in range(2):
                        for st in range(2):
                            nc.tensor.matmul(
                                m_ps[d][:, sl],
                                s2h_sb[i][:, (st * 2 + d) * P:(st * 2 + d + 1) * P],
                                A[st][:, sl], start=False, stop=(st == 1))
                        nc.scalar.activation(
                            mixed[d][:, sl], m_ps[d][:, sl],
                            AF.Gelu_apprx_tanh,
                            bias=db_sb[:, i * 2 + d:i * 2 + d + 1])
                    if not last and hi == 0:
                        # preload the sqrt table for the next layer's stats
                        # in the scalar-idle window right after the gelus
                        nc.scalar.activation(scrap[:], eps11[:], AF.Sqrt)
                    for d2 in range(2):
                        for d in range(2):
                            nc.tensor.matmul(
                                d_ps[d2][:, sl],
                                outp_sb[i][:, (d * 2 + d2) * P:(d * 2 + d2 + 1) * P],
                                mixed[d][:, sl], start=(d == 0),
                                stop=(d == 1))
                        dst = hT[d2] if not last else hsT[d2]
                        if use_opb:
                            nc.vector.scalar_tensor_tensor(
                                dst[:, sl], d_ps[d2][:, sl], 1.0,
                                hT[d2][:, sl].bitcast(f32),
                                op0=OP.bypass, op1=OP.add)
                            nc.vector.tensor_scalar(
                                dst[:, sl], dst[:, sl].bitcast(f32)
                                if not last else dst[:, sl],
                                ob_sb[:, i * 2 + d2:i * 2 + d2 + 1], None,
                                op0=OP.add)
                        else:
                            nc.vector.tensor_tensor(dst[:, sl],
                                                    d_ps[d2][:, sl],
                                                    hT[d2][:, sl].bitcast(f32),
                                                    op=OP.add)
                if dbg and i == dbg_l:
                    nc.sync.dma_start(dbg_d[5], mixed[0].bitcast(f32))
                    nc.sync.dma_start(dbg_d[6], mixed[1].bitcast(f32))
                    nc.sync.dma_start(dbg_d[10], A[0][:].bitcast(f32))

            if dbg:
                for d2 in range(2):
                    nc.sync.dma_start(dbg2_d[d2], hsT[d2][:])
            # ---- output projection: out[t, v] = hsT[:, t] . outwt[:, v] ----
            # bf16 output, staged in SBUF, stores spread over 5 DMA queues
            for mt in range(NT):
                for vg in range(NVC // 4):
                    st_t = stg.tile([P, 4 * VC], bf16, tag="stg", bufs=8)
                    for vs in range(4):
                        vc = vg * 4 + vs
                        p_ps = ps.tile([P, VC], f32, tag="pp", bufs=8, name="p_ps")
                        for d in range(2):
                            nc.tensor.matmul(p_ps[:], hsT[d][:, mt * P:(mt + 1) * P],
                                             outwt_sb[d][:, vc * VC:(vc + 1) * VC],
                                             start=(d == 0),
                                             stop=(d == 1 and not use_outb))
                        if use_outb:
                            nc.tensor.matmul(p_ps[:], ones1_sb[:],
                                             outb_sb[:, vc * VC:(vc + 1) * VC],
                                             start=False, stop=True)
                        if vc % 8 < 5:
                            nc.vector.tensor_copy(st_t[:, vs * VC:(vs + 1) * VC], p_ps[:])
                        else:
                            nc.scalar.activation(st_t[:, vs * VC:(vs + 1) * VC], p_ps[:],
                                                 AF.Identity)
                    # 64-partition stores: a 128-partition DMA fans out to
                    # two HW-DGE queues whose completion tracking is unsound
                    # under buffer reuse (see tile.py optimize_sems note);
                    # 64-row transfers stay on one queue each.
                    nsh = 2
                    for sh in range(nsh):
                        w = 4 * VC // nsh
                        for ph in range(2):
                            eng = (nc.sync, nc.gpsimd)[
                                (mt * 8 + vg * nsh + sh + ph) % 2]
                            eng.dma_start(
                                out_d[mt * P + ph * 64:mt * P + (ph + 1) * 64,
                                      vg * 4 * VC + sh * w:vg * 4 * VC + (sh + 1) * w],
                                st_t[ph * 64:(ph + 1) * 64, sh * w:(sh + 1) * w])

    nc.compile()
    _cache[key] = nc
    return nc


def _pack_lhsT(w):
    """w: [M, K] weight for out = w @ x. Returns [128, (K/128)*(M/128)*128]
    lhsT pack; block b = kt*nmt + mt holds lhsT[kt*128+p, mt*128+m]."""
    M, K = w.shape
    lhsT = np.ascontiguousarray(w.T)                       # [K, M]
    t = lhsT.reshape(K // P, P, M // P, P)                 # [kt, p, mt, m]
    return np.ascontiguousarray(t.transpose(1, 0, 2, 3).reshape(P, -1))


def kernel(**inputs):
    import ml_dtypes
    xs = {k: np.asarray(v) for k, v in inputs.items()}
    tokens = xs["tokens"].astype(np.int32)
    token_embed = xs["token_embed"].astype(np.float32)
    pos_embed = xs["pos_embed"].astype(np.float32)
    in_to_state = xs["in_to_state"].astype(np.float64)
    state_to_hidden = xs["state_to_hidden"].astype(np.float64)
    direct = xs["direct"].astype(np.float64)
    a_diag = xs["a_diag"].astype(np.float64)
    g_diag = xs["g_diag"].astype(np.float64)
    dtp = xs["dt"].astype(np.float64)
    ln_w = xs["ln_w"].astype(np.float64)
    ln_b = xs["ln_b"].astype(np.float64)
    outp_W = xs["outp_W"].astype(np.float64)
    outp_b = xs["outp_b"].astype(np.float32)
    out_W = xs["out_W"].astype(np.float32)
    out_b = xs["out_b"].astype(np.float32)

    def softplus(x):
        return np.logaddexp(0.0, x)

    dt_e = softplus(dtp) + 1e-4
    coeff = np.exp(-softplus(g_diag) * dt_e) * np.cos(a_diag * dt_e)   # [NB, S]
    cdecay = coeff ** T                                                 # [NB, S]
    # c^(t+1) tables for the carry correction, [NB, 2, P, T]
    tpow = np.arange(1, T + 1, dtype=np.float64)
    cpow = coeff.reshape(NB, 2, P, 1) ** tpow.reshape(1, 1, 1, T)
    trev = np.arange(T - 1, -1, -1, dtype=np.float64)
    crev = coeff.reshape(NB, 2, P, 1) ** trev.reshape(1, 1, 1, T)

    # packed weights (shared across cores)
    winp = [in_to_state[i] * ln_w[i][None, :] for i in range(NB)]
    win_pack = np.stack([_pack_lhsT(winp[i]) for i in range(NB)])
    s2h_pack = np.stack([_pack_lhsT(state_to_hidden[i]) for i in range(NB)])
    outp_pack = np.stack([_pack_lhsT(outp_W[i]) for i in range(NB)])
    dprime = direct * ln_w                                              # [NB, D]
    ddiag_pack = np.ascontiguousarray(np.concatenate(
        [np.stack([np.diag(dprime[i, d * P:(d + 1) * P]) for d in range(2)],
                  axis=1).reshape(P, 2 * P)[None] for i in range(NB)]))
    wl = np.concatenate([win_pack, s2h_pack, outp_pack, ddiag_pack],
                        axis=2).astype(np.float32)
    cpw = np.concatenate([
        cpow.transpose(0, 2, 1, 3).reshape(NB, P, 2 * T),
        crev.transpose(0, 2, 1, 3).reshape(NB, P, 2 * T)], axis=2).astype(np.float32)

    # rank-1 LN-fold rows: -w1[s] = -sum_d win'[s,d]; -dprime rows
    w1 = np.stack([winp[i].sum(axis=1) for i in range(NB)])             # [NB, S]
    rows = np.concatenate([(-w1).reshape(1, NB * S),
                           (-dprime).reshape(1, NB * D)],
                          axis=1).astype(np.float32)

    outwt_pack = np.ascontiguousarray(out_W.T.reshape(2, P, V))
    outwt_bf16 = outwt_pack.astype(ml_dtypes.bfloat16)
    ubias = np.stack([in_to_state[i] @ ln_b[i] for i in range(NB)])     # [NB, S]
    dbias = direct * ln_b                                               # [NB, D]

    def cols(v):  # [NB, 256] -> [128, NB*2] with col (i*2+half)
        return np.ascontiguousarray(
            v.reshape(NB, 2, P).transpose(2, 0, 1).reshape(P, NB * 2)).astype(np.float32)

    use_outb = bool(np.any(out_b != 0.0))
    use_ubias = bool(np.any(np.abs(ubias) > 0.0))
    use_opb = bool(np.any(outp_b != 0.0))

    cblob = np.concatenate([
        np.eye(P, dtype=np.float32), cols(coeff), cols(dbias),
        cols(np.broadcast_to(outp_b, (NB, D)).astype(np.float64))], axis=1)

    base = dict(
        tok_tab=token_embed, cblob_in=cblob, rows_in=rows,
        onesT=np.ones((1, T), ml_dtypes.bfloat16),
        ubias_in=ubias.reshape(1, NB * 2 * P).astype(ml_dtypes.bfloat16),
        wl_in=wl, cpw_in=cpw,
        outwt_in=outwt_bf16,
        outb_in=out_b.reshape(1, V).astype(ml_dtypes.bfloat16),
        ones8=np.ones((8, 1), np.float32),
    )

    # per-core wm weights: wm[j, s] = cdecay[s]^(k-1-j) for sender rank j < k
    def wm_rank(k):
        wm = np.zeros((8, NB, S), np.float64)
        for j in range(k):
            wm[j] = cdecay ** (k - 1 - j)
        return wm

    in_maps = []
    for k in range(NC):
        sl = slice(k * T, (k + 1) * T)
        tk_ = tokens[sl].reshape(NT, P).T.copy()           # [128, NT]
        # pre-transposed positional embeddings: [D-half rows, T]
        pe = pos_embed[sl]                                  # [T, D]
        posT = np.ascontiguousarray(
            pe.T.reshape(2, P, T).transpose(1, 0, 2).reshape(P, 2 * T))
        in_maps.append(dict(
            base, tok_idx=tk_, posT_in=posT.astype(np.float32),
            wmat_in=wm_rank(k).reshape(8, NB * S).astype(np.float32)))

    trace = bool(os.environ.get("BASS_KERNEL_TRACE"))
    tk = {}
    if os.environ.get("BASS_TRACE_ALL_CORES"):
        tk["trace_cores"] = list(range(NC))
    res = run_bass_kernel_spmd(_build((use_outb, use_ubias, use_opb)),
                               in_maps, core_ids=list(range(NC)),
                               trace=trace, **tk)

    kernel.last_results = res
    if trace:
        kernel.last_exec_time_ns = res.exec_time_ns
    return np.concatenate(
        [np.asarray(res.results[k]["out"]).astype(np.float32) for k in range(NC)],
        axis=0)


# revision 38
# speedup vs baseline: 1.0811x; 1.0811x over previous
"""LocalLinOSS Trainium2 kernel — 8-core SPMD, sequence-sharded, hT-resident.

Model structure (reference): embedding lookup -> 4 sequential blocks; within a
timestep, block i reads the running hidden h (reset to x_t each step), so the
only cross-time recurrence is the per-block diagonal state
    ns_t = coeff (.) ns_{t-1} + in_to_state @ LN(h_t).
The model therefore decomposes into 4 sequential layer passes over the whole
sequence, each = big matmuls over L (parallel) + a first-order linear scan
(hardware tensor_tensor_scan), followed by the [L,D]@[D,V] output projection.

Sharding: L=4096 split into 8 chunks of T=512 (one per core). Per layer, each
core computes a local scan with zero initial state, exchanges the 8 chunk
final states (1KB payload) via ncfw AllGather, combines them into its carry-in
using host-precomputed decay powers, and applies the carry as a correction
through the s2h matmul: s2h @ ns = s2h @ ns_local + s2h @ A where
A[s,t] = c_s^{t+1} * carry_s. The two s2h @ ns_local matmuls and the scans are
issued before the exchange so they overlap it.

Layout: the hidden state lives TRANSPOSED the whole time: hT[d] = [128 D-half
rows, T=512 time cols] f32r, d in {0,1}. This kills all per-layer transposes:
 - LN is folded into the matmuls: u = (win @ hT  -  w1 (x) mu_row) (.) r_row,
   where mu/r = per-timestep mean/rstd. The rank-1 mean-subtract term is a
   K=1 matmul accumulated into the same PSUM; the rstd scale rides the
   PSUM->SBUF vector op (r broadcast across partitions via gpsimd
   partition_broadcast — DVE ops reject partition-stride-0 APs and can read
   only one PSUM operand). Same fold for the ddiag (direct) term.
 - LN stats come from PE column-sums (ones-column f32r matmuls over hT and
   hT^2); rstd = reciprocal_approx_fast(Sqrt(var+eps)) — scalar Rsqrt is
   gated in bass, and the Sqrt/Gelu activation tables don't coexist, so
   dummy activations hoist the 1.28us table swaps into scalar-idle windows.
 - delta = outp_W' @ mixed comes out of PSUM already in [D, T] layout, so the
   residual is a plain vector add (bf16-cast fold on the last layer, which is
   processed in T-halves so the vocab projection starts on the first half).
Producers feeding f32r matmuls must write f32r-rounded APs (BIR verifier
enforces it) — hence native-f32r tiles with .bitcast(f32) reads on DVE ops.

Output projection: per-core [T,512]x[256,V] in bf16 (fp32 accumulate), bf16
stores. Stores are issued as 64-partition transfers: a 128-partition DMA fans
out across two HW-DGE queues whose completion tracking is unsound under
buffer reuse (see the optimize_sems note in tile.py; observed as partitions
64-127 corrupting under schedule-dependent races), and rotate over the
sync/gpsimd queues only — scalar-queue stores hit the same corruption first.

Timing notes (measured on this terminal): the ncfw first-collective wall is
~76-92us per NEFF execution (a minimal 1-AllGather kernel runs 92us), so AG0
completes at ~77-86us no matter what; later AGs cost ~5-7us each incl skew.
The part is power-throttled (ham windows cap PE util at 50-81% for most of
the run), so the PE sustains only ~1.15-1.2 GHz: keep-warm filler matmuls
are net NEGATIVE (they burn throttle budget) and were removed; cost-model
full-speed estimates do not apply. Run-to-run variance is +-7us (wall,
throttle phase, inter-core skew). HW exec time ~215-228us vs 230us for the
previous (transpose-heavy, warm-filled) kernel.
"""
import os
import sys
sys.path.insert(0, "/opt/trn_rl_repo")
import numpy as np
import concourse.bass as bass
import concourse.bacc as bacc
import concourse.mybir as mybir
import concourse.tile as tile
from concourse.bass_utils import run_bass_kernel_spmd

L, D, S, NB, V = 4096, 256, 256, 4, 8000
NC = 8
T = L // NC            # 512 timesteps per core
P = 128
NT = T // P            # 4 T-tiles per core
NVC = 16               # projection V chunks
VC = V // NVC          # 500
f32 = mybir.dt.float32
f32r = mybir.dt.float32r
bf16 = mybir.dt.bfloat16
i32 = mybir.dt.int32
AF = mybir.ActivationFunctionType
OP = mybir.AluOpType
AX = mybir.AxisListType

WARM0 = 8              # startup PE-warm matmuls (512-col bf16)
NWARM = 0              # exchange-wait PE-warm matmuls
L0WARM = 0            # extra warm filling the first-collective wall
WGELU = 0              # fill the gelu wait between A-matmuls and residual
WRES = 0               # fill the stats wait at end of layer

_cache = {}


def _build(flags):
    key = flags
    if (nc_cached := _cache.get(key)) is not None:
        return nc_cached
    use_outb, use_ubias, use_opb = flags
    dbg = bool(os.environ.get("BASS_DBG_DUMP"))
    dbg_l = int(os.environ.get("BASS_DBG_LAYER", "0"))
    key = flags + (dbg, dbg_l)
    nc = bacc.Bacc("TRN2", target_bir_lowering=False, debug=False,
                   enable_asserts=True, num_devices=NC)

    def din(name, shape, dtype=f32):
        return nc.dram_tensor(name, shape, dtype, kind="ExternalInput").ap()

    tok_idx = din("tok_idx", [P, NT], i32)
    tok_tab = din("tok_tab", [V, D])
    posT_in = din("posT_in", [P, 2 * T])   # pre-transposed pos embed, 2 halves
    # consts blob: id_f32 (P) | coef (NB*2) | dbias (NB*2) | opb (NB*2)
    cblob_in = din("cblob_in", [P, P + NB * 6])
    # rank-1 rows for the LN fold: -w1 | -dprime, per (layer, half): [1, NB*4*P]
    rows_in = din("rows_in", [1, NB * 4 * P])
    ubias_in = din("ubias_in", [1, NB * 2 * P], bf16)  # W_in' @ ln_b lhsT rows
    onesT = din("onesT", [1, T], bf16)
    ones8 = din("ones8", [8, 1])
    wmat_in = din("wmat_in", [8, NB * S])      # per-core carry weights
    # per-layer weight blob: win (4P) | s2h (4P) | outp (4P) | ddiag (2P), f32
    wl_in = din("wl_in", [NB, P, 14 * P])
    # per-layer decay powers: cpow st0,st1 | crev st0,st1  (f32)
    cpw_in = din("cpw_in", [NB, P, 4 * T])
    outwt_in = din("outwt_in", [2, P, V], bf16)
    outb_in = din("outb_in", [1, V], bf16)
    out_d = nc.dram_tensor("out", [T, V], bf16, kind="ExternalOutput").ap()
    if dbg:
        dbg_d = nc.dram_tensor("dbg", [12, P, T], f32,
                               kind="ExternalOutput").ap()
        dbg2_d = nc.dram_tensor("dbg2", [2, P, T], bf16,
                                kind="ExternalOutput").ap()

    with tile.TileContext(nc) as tc:
        with tc.tile_pool(name="const", bufs=1) as cst, \
             tc.tile_pool(name="wts", bufs=1) as wts, \
             tc.tile_pool(name="work", bufs=1) as wk, \
             tc.tile_pool(name="lay", bufs=1) as lay, \
             tc.tile_pool(name="psum", bufs=1, space="PSUM") as ps, \
             tc.tile_pool(name="stage", bufs=1) as stg, \
             tc.tile_pool(name="dram", bufs=1, space="DRAM") as dram:

            # ---- PE self-warm source: memset, no DMA dependency ----
            warm = cst.tile([P, T], bf16)
            nc.gpsimd.memset(warm[:], 0.0)
            eps11 = cst.tile([1, 1], f32)
            nc.gpsimd.memset(eps11[:], 1e-5)
            scrap = cst.tile([1, 1], f32)          # dummy act target
            nc.gpsimd.memset(scrap[:], 1.0)

            def warm_mms(n, tag):
                if n == 0:
                    return
                w_ps = ps.tile([P, T], f32, tag="pp", bufs=8, name=f"warm{tag}")
                for w in range(n):
                    nc.tensor.matmul(w_ps[:], warm[:, :P], warm[:],
                                     start=True, stop=True)

            # ---- loads: small control data on sync/scalar; embedding gather
            # gets DMA bandwidth priority, big weight blobs queue behind it ----
            ti_sb = wk.tile([P, NT], i32)
            nc.sync.dma_start(ti_sb[:], tok_idx)
            cb_sb = cst.tile([P, P + NB * 6], f32)
            nc.sync.dma_start(cb_sb[:], cblob_in)
            coef_sb = cb_sb[:, P:P + NB * 2]
            db_sb = cb_sb[:, P + NB * 2:P + NB * 4]
            ob_sb = cb_sb[:, P + NB * 4:P + NB * 6]
            id_r = cst.tile([P, P], f32r)
            nc.sync.dma_start(id_r[:], cblob_in[:, 0:P].bitcast(f32r))
            id_f = cb_sb[:, 0:P]
            posT_sb = wk.tile([P, 2 * T], f32)
            nc.sync.dma_start(posT_sb[:], posT_in)
            rows_sb = cst.tile([1, NB * 4 * P], f32r)
            nc.scalar.dma_start(rows_sb[:], rows_in.bitcast(f32r))
            w1_sb = rows_sb[:, 0:NB * 2 * P]       # -w1 rows (layer, half)
            dp_sb = rows_sb[:, NB * 2 * P:]        # -dprime rows
            ones8_sb = cst.tile([8, 1], f32)
            nc.scalar.dma_start(ones8_sb[:], ones8)
            wm_sb = cst.tile([8, NB * S], f32)
            nc.scalar.dma_start(wm_sb[:], wmat_in)
            if use_ubias:
                ub_sb = cst.tile([1, NB * 2 * P], bf16)
                nc.scalar.dma_start(ub_sb[:], ubias_in)
                onesT_sb = cst.tile([1, T], bf16)
                nc.scalar.dma_start(onesT_sb[:], onesT)
            if use_outb:
                outb_sb = cst.tile([1, V], bf16)
                nc.scalar.dma_start(outb_sb[:], outb_in)
                ones1_sb = cst.tile([1, P], bf16)
                nc.scalar.dma_start(ones1_sb[:], onesT[:, :P])

            # preload the sqrt act table for layer 0's stats; the gelu table
            # gets loaded during layer 0's exchange wait
            nc.scalar.activation(scrap[:], eps11[:], AF.Sqrt)

            # PE warm-up stream: starts as soon as memset lands and keeps the
            # clock ramping while the embedding gather runs
            warm_mms(WARM0, "w0")

            # ---- embedding gather (bandwidth priority) ----
            h0 = wk.tile([P, NT, D], f32)
            for ct in range(NT):
                nc.gpsimd.indirect_dma_start(
                    out=h0[:, ct, :], out_offset=None, in_=tok_tab,
                    in_offset=bass.IndirectOffsetOnAxis(
                        ap=ti_sb[:, ct:ct + 1], axis=0))

            # big weight loads, queued on gpsimd behind the gathers
            wl_sb, cpw_sb = [], []
            for i in range(NB):
                wl_sb.append(wts.tile([P, 14 * P], f32r, name=f"wl{i}"))
                nc.gpsimd.dma_start(wl_sb[i][:], wl_in[i].bitcast(f32r))
                cpw_sb.append(wts.tile([P, 4 * T], f32, name=f"cpw{i}"))
                nc.gpsimd.dma_start(cpw_sb[i][:], cpw_in[i])
            win_sb = [wl_sb[i][:, 0:4 * P] for i in range(NB)]
            s2h_sb = [wl_sb[i][:, 4 * P:8 * P] for i in range(NB)]
            outp_sb = [wl_sb[i][:, 8 * P:12 * P] for i in range(NB)]
            ddiag_sb = [wl_sb[i][:, 12 * P:14 * P] for i in range(NB)]
            outwt_sb = [wts.tile([P, V], bf16, name=f"outwt{d}") for d in range(2)]
            for d in range(2):
                nc.gpsimd.dma_start(outwt_sb[d][:], outwt_in[d])

            # ---- transpose embeddings to [D, T] and add posT ----
            hT = [wk.tile([P, T], f32r, name=f"hT{d}") for d in range(2)]
            ht_ps = [ps.tile([P, T], f32, tag="pp", bufs=8, name=f"ht_ps{d}")
                     for d in range(2)]
            for ct in range(NT):
                for d in range(2):
                    nc.tensor.transpose(ht_ps[d][:, ct * P:(ct + 1) * P],
                                        h0[:, ct, d * P:(d + 1) * P], id_f)
            for d in range(2):
                nc.vector.tensor_tensor(hT[d][:], ht_ps[d][:],
                                        posT_sb[:, d * T:(d + 1) * T], op=OP.add)

            # a [128,1] f32r ones column for column-sum matmuls
            onescol = cst.tile([P, 1], f32r)
            nc.gpsimd.memset(onescol[:].bitcast(f32), 1.0)

            hsT = [None, None]

            # ---- 4 sequential layer passes ----
            for i in range(NB):
                last = i == NB - 1
                # 1. stats: column sums of hT and hT^2 via ones-matmuls
                sq0 = lay.tile([P, T], f32r, tag="sq0")
                sq1 = lay.tile([P, T], f32r, tag="sq1")
                nc.scalar.activation(sq0[:], hT[0][:].bitcast(f32), AF.Square)
                nc.vector.tensor_tensor(sq1[:], hT[1][:].bitcast(f32),
                                        hT[1][:].bitcast(f32), op=OP.mult)
                cs_ps = ps.tile([1, T], f32, tag="pp", bufs=8, name="cs_ps")
                cq_ps = ps.tile([1, T], f32, tag="pp", bufs=8, name="cq_ps")
                for d in range(2):
                    nc.tensor.matmul(cs_ps[:], onescol[:], hT[d][:],
                                     start=(d == 0), stop=(d == 1))
                nc.tensor.matmul(cq_ps[:], onescol[:], sq0[:],
                                 start=True, stop=False)
                nc.tensor.matmul(cq_ps[:], onescol[:], sq1[:],
                                 start=False, stop=True)
                # rows: mean, veps, r=rsqrt(veps), rmu=r*mean
                mean = lay.tile([1, T], f32r, tag="mean")
                ms = lay.tile([1, T], f32, tag="ms")
                veps = lay.tile([1, T], f32, tag="veps")
                sd = lay.tile([1, T], f32, tag="sd")
                r_row = lay.tile([1, T], f32, tag="r_row")
                nc.vector.tensor_scalar_mul(mean[:], cs_ps[:], 1.0 / D)
                nc.scalar.activation(ms[:], cs_ps[:], AF.Square,
                                     scale=1.0 / D)
                nc.vector.scalar_tensor_tensor(veps[:], cq_ps[:], 1.0 / D,
                                               ms[:], op0=OP.mult,
                                               op1=OP.subtract)
                # rstd = recip(sqrt(var+eps)): sqrt on scalar (table preloaded
                # by a dummy), single-instruction DVE approx reciprocal
                nc.scalar.activation(sd[:], veps[:], AF.Sqrt, bias=eps11[:])
                nc.vector.reciprocal_approx_fast(r_row[:], sd[:])
                # broadcast r_row to all partitions in SBUF (DVE ops can't
                # take partition-stride-0 operands, nor two PSUM inputs)
                r_sb = lay.tile([P, T], f32, tag="r_sb")
                nc.gpsimd.partition_broadcast(r_sb[:], r_row[:])

                # 2. u = (win @ hT - w1 (x) rmu) (.) r_row ; the chunk summary
                # (last scan state) = sum_k crev_k u_k via fused accumulate
                u = [lay.tile([P, T], f32, tag=f"u{st}", name=f"u{st}")
                     for st in range(2)]
                last2 = lay.tile([P, 4], f32, tag="last2")
                scrT = sq0  # dead after the cq colsum; reuse as accum target
                u_ps = [None, None]
                for st in range(2):
                    u_ps[st] = ps.tile([P, T], f32, tag="pp", bufs=8,
                                       name=f"u_ps{st}")
                    for kt in range(2):
                        nc.tensor.matmul(
                            u_ps[st][:],
                            win_sb[i][:, (kt * 2 + st) * P:(kt * 2 + st + 1) * P],
                            hT[kt][:], start=(kt == 0), stop=False)
                    if use_ubias:
                        nc.tensor.matmul(
                            u_ps[st][:],
                            ub_sb[:, (i * 2 + st) * P:(i * 2 + st + 1) * P],
                            onesT_sb[:], start=False, stop=False)
                    nc.tensor.matmul(
                        u_ps[st][:],
                        w1_sb[:, (i * 2 + st) * P:(i * 2 + st + 1) * P],
                        mean[:], start=False, stop=True)
                    # scale by r on the way out of PSUM
                    nc.vector.tensor_tensor(
                        u[st][:], u_ps[st][:], r_sb[:], op=OP.mult)
                    nc.vector.scalar_tensor_tensor(
                        scrT[:], u[st][:], 1.0,
                        cpw_sb[i][:, (2 + st) * T:(3 + st) * T],
                        op0=OP.bypass, op1=OP.mult,
                        accum_out=last2[:, st:st + 1])
                if dbg and i == dbg_l:
                    nc.sync.dma_start(dbg_d[0], r_sb[:])
                    nc.sync.dma_start(dbg_d[1], u[0][:])
                    nc.sync.dma_start(dbg_d[2], u[1][:])
                # 3. export chunk-final states (AllGather via shared DRAM)
                lt_ps = ps.tile([2, P], f32, tag="pp", bufs=8, name="lt_ps")
                nc.tensor.transpose(lt_ps[:], last2[:, 0:2], id_f)
                exp_sb = lay.tile([2, P], f32, tag="exp")
                nc.scalar.activation(exp_sb[:], lt_ps[:], AF.Identity)
                ag_in = dram.tile([2, P], f32, name=f"ag_in{i}")
                ag_out = dram.tile([NC, 2, P], f32, name=f"ag_out{i}",
                                   addr_space="Shared")
                nc.scalar.dma_start(ag_in[:], exp_sb[:])
                # swap the act table back to gelu while the scalar engine
                # would otherwise idle in the exchange wait
                nc.scalar.activation(scrap[:], eps11[:], AF.Gelu_apprx_tanh)
                nc.gpsimd.collective_compute(
                    "AllGather", OP.bypass, replica_groups=[list(range(NC))],
                    ins=[ag_in[:]], outs=[ag_out[:]])
                # 4. overlap the AG: local scans + carry-independent partials
                ns1 = [lay.tile([P, T], f32r, tag=f"ns1{st}", name=f"ns1{st}")
                       for st in range(2)]
                for st in range(2):
                    cb = coef_sb[:, i * 2 + st:i * 2 + st + 1].to_broadcast((P, T))
                    nc.vector.tensor_tensor_scan(ns1[st][:], cb,
                                                 u[st][:], 0.0,
                                                 op0=OP.mult, op1=OP.add)
                # dd = (ddiag @ hT - dprime (x) rmu) (.) r_row
                dd = [lay.tile([P, T], f32r, tag=f"dd{d}", name=f"dd{d}")
                      for d in range(2)]
                for d in range(2):
                    dd_ps = ps.tile([P, T], f32, tag="pp", bufs=8, name=f"dd_ps{d}")
                    nc.tensor.matmul(dd_ps[:],
                                     ddiag_sb[i][:, d * P:(d + 1) * P],
                                     hT[d][:], start=True, stop=False)
                    nc.tensor.matmul(dd_ps[:],
                                     dp_sb[:, (i * 2 + d) * P:(i * 2 + d + 1) * P],
                                     mean[:], start=False, stop=True)
                    nc.vector.tensor_tensor(dd[d][:], dd_ps[:],
                                            r_sb[:], op=OP.mult)
                if dbg and i == dbg_l:
                    nc.sync.dma_start(dbg_d[3], ns1[0][:].bitcast(f32))
                    nc.sync.dma_start(dbg_d[4], ns1[1][:].bitcast(f32))
                    nc.sync.dma_start(dbg_d[9], dd[0][:].bitcast(f32))
                # m_ps = s2h @ ns1 + dd (via identity matmul); left open for
                # the post-carry s2h @ A accumulate
                m_ps = [None, None]
                for d in range(2):
                    m_ps[d] = ps.tile([P, T], f32, tag="pp", bufs=8, name=f"m_ps{d}")
                    for st in range(2):
                        nc.tensor.matmul(m_ps[d][:],
                                         s2h_sb[i][:, (st * 2 + d) * P:(st * 2 + d + 1) * P],
                                         ns1[st][:], start=(st == 0), stop=False)
                    nc.tensor.matmul(m_ps[d][:], id_r[:], dd[d][:],
                                     start=False, stop=False)
                # PE keep-warm through the exchange wait
                warm_mms(NWARM + (L0WARM if i == 0 else 0), f"c{i}")
                # 5. combine the gathered states into the carry
                A = dd  # dd dead after its id-matmul into m_ps
                gath = lay.tile([8, S], f32, tag="gath")
                nc.scalar.dma_start(gath[:],
                                    ag_out[:].rearrange("c a b -> c (a b)"))
                q = lay.tile([8, S], f32, tag="q")
                nc.vector.tensor_tensor(q[:], wm_sb[:, i * S:(i + 1) * S],
                                        gath[:], op=OP.mult)
                for st in range(2):
                    c_ps = ps.tile([P, 1], f32, tag="pp", bufs=8,
                                   name=f"c_ps{st}")
                    nc.tensor.matmul(c_ps[:], q[:, st * P:(st + 1) * P],
                                     ones8_sb[:], start=True, stop=True)
                    nc.vector.tensor_scalar_mul(
                        A[st][:],
                        cpw_sb[i][:, st * T:(st + 1) * T], c_ps[:, 0:1])
                # 6.-8. finish mixed, delta, residual. On the last layer,
                # process T-halves so the vocab projection can start on the
                # first half while the second is still in flight.
                # (sq tiles are dead after the colsums/accum and are f32r
                # throughout, so the gelu output reuses them.)
                mixed = [sq0[:], sq1[:]]
                d_ps = [None, None]
                for d2 in range(2):
                    d_ps[d2] = ps.tile([P, T], f32, tag="pp", bufs=8,
                                       name=f"d_ps{d2}")
                if last:
                    for d2 in range(2):
                        hsT[d2] = wk.tile([P, T], bf16, name=f"hsT{d2}")
                halves = [slice(0, T)] if not last else \
                    [slice(0, T // 2), slice(T // 2, T)]
                for hi, sl in enumerate(halves):
                    for d in range(2):
                        for st in range(2):
                            nc.tensor.matmul(
                                m_ps[d][:, sl],
                                s2h_sb[i][:, (st * 2 + d) * P:(st * 2 + d + 1) * P],
                                A[st][:, sl], start=False, stop=(st == 1))
                        nc.scalar.activation(
                            mixed[d][:, sl], m_ps[d][:, sl],
                            AF.Gelu_apprx_tanh,
                            bias=db_sb[:, i * 2 + d:i * 2 + d + 1])
                    if not last and hi == 0:
                        # preload the sqrt table for the next layer's stats
                        # in the scalar-idle window right after the gelus
                        nc.scalar.activation(scrap[:], eps11[:], AF.Sqrt)
                    for d2 in range(2):
                        for d in range(2):
                            nc.tensor.matmul(
                                d_ps[d2][:, sl],
                                outp_sb[i][:, (d * 2 + d2) * P:(d * 2 + d2 + 1) * P],
                                mixed[d][:, sl], start=(d == 0),
                                stop=(d == 1))
                        dst = hT[d2] if not last else hsT[d2]
                        if use_opb:
                            nc.vector.scalar_tensor_tensor(
                                dst[:, sl], d_ps[d2][:, sl], 1.0,
                                hT[d2][:, sl].bitcast(f32),
                                op0=OP.bypass, op1=OP.add)
                            nc.vector.tensor_scalar(
                                dst[:, sl], dst[:, sl].bitcast(f32)
                                if not last else dst[:, sl],
                                ob_sb[:, i * 2 + d2:i * 2 + d2 + 1], None,
                                op0=OP.add)
                        else:
                            nc.vector.tensor_tensor(dst[:, sl],
                                                    d_ps[d2][:, sl],
                                                    hT[d2][:, sl].bitcast(f32),
                                                    op=OP.add)
                if dbg and i == dbg_l:
                    nc.sync.dma_start(dbg_d[5], mixed[0].bitcast(f32))
                    nc.sync.dma_start(dbg_d[6], mixed[1].bitcast(f32))
                    nc.sync.dma_start(dbg_d[10], A[0][:].bitcast(f32))

            if dbg:
                for d2 in range(2):
                    nc.sync.dma_start(dbg2_d[d2], hsT[d2][:])
            # ---- output projection: out[t, v] = hsT[:, t] . outwt[:, v] ----
            # bf16 output, staged in SBUF, stores spread over 5 DMA queues
            for mt in range(NT):
                for vg in range(NVC // 4):
                    st_t = stg.tile([P, 4 * VC], bf16, tag="stg", bufs=8)
                    for vs in range(4):
                        vc = vg * 4 + vs
                        p_ps = ps.tile([P, VC], f32, tag="pp", bufs=8, name="p_ps")
                        for d in range(2):
                            nc.tensor.matmul(p_ps[:], hsT[d][:, mt * P:(mt + 1) * P],
                                             outwt_sb[d][:, vc * VC:(vc + 1) * VC],
                                             start=(d == 0),
                                             stop=(d == 1 and not use_outb))
                        if use_outb:
                            nc.tensor.matmul(p_ps[:], ones1_sb[:],
                                             outb_sb[:, vc * VC:(vc + 1) * VC],
                                             start=False, stop=True)
                        if vc % 8 < 5:
                            nc.vector.tensor_copy(st_t[:, vs * VC:(vs + 1) * VC], p_ps[:])
                        else:
                            nc.scalar.activation(st_t[:, vs * VC:(vs + 1) * VC], p_ps[:],
                                                 AF.Identity)
                    # 64-partition stores: a 128-partition DMA fans out to
                    # two HW-DGE queues whose completion tracking is unsound
                    # under buffer reuse (see tile.py optimize_sems note);
                    # 64-row transfers stay on one queue each.
                    nsh = 1
                    for sh in range(nsh):
                        w = 4 * VC // nsh
                        for ph in range(2):
                            eng = (nc.sync, nc.gpsimd)[
                                (mt * 16 + vg * 4 + sh * 2 + ph) % 2]
                            eng.dma_start(
                                out_d[mt * P + ph * 64:mt * P + (ph + 1) * 64,
                                      vg * 4 * VC + sh * w:vg * 4 * VC + (sh + 1) * w],
                                st_t[ph * 64:(ph + 1) * 64, sh * w:(sh + 1) * w])

    nc.compile()
    _cache[key] = nc
    return nc


def _pack_lhsT(w):
    """w: [M, K] weight for out = w @ x. Returns [128, (K/128)*(M/128)*128]
    lhsT pack; block b = kt*nmt + mt holds lhsT[kt*128+p, mt*128+m]."""
    M, K = w.shape
    lhsT = np.ascontiguousarray(w.T)                       # [K, M]
    t = lhsT.reshape(K // P, P, M // P, P)                 # [kt, p, mt, m]
    return np.ascontiguousarray(t.transpose(1, 0, 2, 3).reshape(P, -1))


def kernel(**inputs):
    import ml_dtypes
    xs = {k: np.asarray(v) for k, v in inputs.items()}
    tokens = xs["tokens"].astype(np.int32)
    token_embed = xs["token_embed"].astype(np.float32)
    pos_embed = xs["pos_embed"].astype(np.float32)
    in_to_state = xs["in_to_state"].astype(np.float64)
    state_to_hidden = xs["state_to_hidden"].astype(np.float64)
    direct = xs["direct"].astype(np.float64)
    a_diag = xs["a_diag"].astype(np.float64)
    g_diag = xs["g_diag"].astype(np.float64)
    dtp = xs["dt"].astype(np.float64)
    ln_w = xs["ln_w"].astype(np.float64)
    ln_b = xs["ln_b"].astype(np.float64)
    outp_W = xs["outp_W"].astype(np.float64)
    outp_b = xs["outp_b"].astype(np.float32)
    out_W = xs["out_W"].astype(np.float32)
    out_b = xs["out_b"].astype(np.float32)

    def softplus(x):
        return np.logaddexp(0.0, x)

    dt_e = softplus(dtp) + 1e-4
    coeff = np.exp(-softplus(g_diag) * dt_e) * np.cos(a_diag * dt_e)   # [NB, S]
    cdecay = coeff ** T                                                 # [NB, S]
    # c^(t+1) tables for the carry correction, [NB, 2, P, T]
    tpow = np.arange(1, T + 1, dtype=np.float64)
    cpow = coeff.reshape(NB, 2, P, 1) ** tpow.reshape(1, 1, 1, T)
    trev = np.arange(T - 1, -1, -1, dtype=np.float64)
    crev = coeff.reshape(NB, 2, P, 1) ** trev.reshape(1, 1, 1, T)

    # packed weights (shared across cores)
    winp = [in_to_state[i] * ln_w[i][None, :] for i in range(NB)]
    win_pack = np.stack([_pack_lhsT(winp[i]) for i in range(NB)])
    s2h_pack = np.stack([_pack_lhsT(state_to_hidden[i]) for i in range(NB)])
    outp_pack = np.stack([_pack_lhsT(outp_W[i]) for i in range(NB)])
    dprime = direct * ln_w                                              # [NB, D]
    ddiag_pack = np.ascontiguousarray(np.concatenate(
        [np.stack([np.diag(dprime[i, d * P:(d + 1) * P]) for d in range(2)],
                  axis=1).reshape(P, 2 * P)[None] for i in range(NB)]))
    wl = np.concatenate([win_pack, s2h_pack, outp_pack, ddiag_pack],
                        axis=2).astype(np.float32)
    cpw = np.concatenate([
        cpow.transpose(0, 2, 1, 3).reshape(NB, P, 2 * T),
        crev.transpose(0, 2, 1, 3).reshape(NB, P, 2 * T)], axis=2).astype(np.float32)

    # rank-1 LN-fold rows: -w1[s] = -sum_d win'[s,d]; -dprime rows
    w1 = np.stack([winp[i].sum(axis=1) for i in range(NB)])             # [NB, S]
    rows = np.concatenate([(-w1).reshape(1, NB * S),
                           (-dprime).reshape(1, NB * D)],
                          axis=1).astype(np.float32)

    outwt_pack = np.ascontiguousarray(out_W.T.reshape(2, P, V))
    outwt_bf16 = outwt_pack.astype(ml_dtypes.bfloat16)
    ubias = np.stack([in_to_state[i] @ ln_b[i] for i in range(NB)])     # [NB, S]
    dbias = direct * ln_b                                               # [NB, D]

    def cols(v):  # [NB, 256] -> [128, NB*2] with col (i*2+half)
        return np.ascontiguousarray(
            v.reshape(NB, 2, P).transpose(2, 0, 1).reshape(P, NB * 2)).astype(np.float32)

    use_outb = bool(np.any(out_b != 0.0))
    use_ubias = bool(np.any(np.abs(ubias) > 0.0))
    use_opb = bool(np.any(outp_b != 0.0))

    cblob = np.concatenate([
        np.eye(P, dtype=np.float32), cols(coeff), cols(dbias),
        cols(np.broadcast_to(outp_b, (NB, D)).astype(np.float64))], axis=1)

    base = dict(
        tok_tab=token_embed, cblob_in=cblob, rows_in=rows,
        onesT=np.ones((1, T), ml_dtypes.bfloat16),
        ubias_in=ubias.reshape(1, NB * 2 * P).astype(ml_dtypes.bfloat16),
        wl_in=wl, cpw_in=cpw,
        outwt_in=outwt_bf16,
        outb_in=out_b.reshape(1, V).astype(ml_dtypes.bfloat16),
        ones8=np.ones((8, 1), np.float32),
    )

    # per-core wm weights: wm[j, s] = cdecay[s]^(k-1-j) for sender rank j < k
    def wm_rank(k):
        wm = np.zeros((8, NB, S), np.float64)
        for j in range(k):
            wm[j] = cdecay ** (k - 1 - j)
        return wm

    in_maps = []
    for k in range(NC):
        sl = slice(k * T, (k + 1) * T)
        tk_ = tokens[sl].reshape(NT, P).T.copy()           # [128, NT]
        # pre-transposed positional embeddings: [D-half rows, T]
        pe = pos_embed[sl]                                  # [T, D]
        posT = np.ascontiguousarray(
            pe.T.reshape(2, P, T).transpose(1, 0, 2).reshape(P, 2 * T))
        in_maps.append(dict(
            base, tok_idx=tk_, posT_in=posT.astype(np.float32),
            wmat_in=wm_rank(k).reshape(8, NB * S).astype(np.float32)))

    trace = bool(os.environ.get("BASS_KERNEL_TRACE"))
    tk = {}
    if os.environ.get("BASS_TRACE_ALL_CORES"):
        tk["trace_cores"] = list(range(NC))
    res = run_bass_kernel_spmd(_build((use_outb, use_ubias, use_opb)),
                               in_maps, core_ids=list(range(NC)),
                               trace=trace, **tk)

    kernel.last_results = res
    if trace:
        kernel.last_exec_time_ns = res.exec_time_ns
    return np.concatenate(
        [np.asarray(res.results[k]["out"]).astype(np.float32) for k in range(NC)],
        axis=0)
